# revision 1
# baseline (speedup 1.0000x reference)
"""Trainium2 Bass kernel for nn_DecoderLayer (dense transformer decoder layer).

Full inputs in, full output out. Internally sharded across 8 NeuronCores:
  core c (0..7): batch b = c//4, group rank r = c%4
  - features permuted so heads are contiguous: perm[h*64+d] = d*16+h
  - self/cross attention: head-parallel, 4 heads per core (256 permuted cols)
  - AllGather of attention outputs over each 4-core group
  - layernorm computed redundantly per group (transposed layout, partition
    reductions via ones-matmuls)
  - MLP + final LN: token-parallel, 256 tokens per core, assembled on host

All activations live in transposed layout [feature, token]; weights are
pre-permuted/sliced/scaled on the host and pre-blocked to [128, 8, w] so each
load is a single DMA with 8-32KB contiguous per-partition lines. Matmuls run
in float32r (full rate on TRN2 for free dim >= 256).
"""
import sys

if "/opt/trn_rl_repo" not in sys.path:
    sys.path.insert(0, "/opt/trn_rl_repo")

import numpy as np

import concourse.bass as bass
import concourse.mybir as mybir
import concourse.tile as tile
from concourse import bacc
from concourse.bass import ds
from concourse.bass_utils import run_bass_kernel_spmd

# ---- problem constants (hardcoded per contract) ----
E = 1024          # embed dim
H = 1024          # hidden dim
NH = 16           # heads
HD = 64           # head dim
B = 2             # batch
P = 1024          # problem (query) length
C = 1024          # context length
NCORES = 8
GSZ = 4           # cores per batch group
HPC = NH // GSZ   # heads per core = 4
SH = HPC * HD     # per-core head shard width = 256
TS = P // GSZ     # per-core token slice for MLP = 256
PB = 128          # partitions
EB = E // PB      # 8 feature blocks

DT = mybir.dt.float32r
F32 = mybir.dt.float32
Exp = mybir.ActivationFunctionType.Exp
Sqrt = mybir.ActivationFunctionType.Sqrt
ADD = mybir.AluOpType.add
MULT = mybir.AluOpType.mult
SUB = mybir.AluOpType.subtract
MAX = mybir.AluOpType.max

_COMPILED = None
TRACE = False      # test harness may flip this


def _build():
    nc = bacc.Bacc(trn_type="TRN2", num_devices=NCORES,
                   target_bir_lowering=False, debug=False)

    def din(name, shape, dt=DT):
        return nc.dram_tensor(name, shape, dt, kind="ExternalInput").ap()

    # big inputs, host pre-blocked to [128, EB, w]
    xT = din("xT", [PB, EB, P])
    ctxT = din("ctxT", [PB, EB, C])
    wq = din("wq", [PB, EB, SH]); wk = din("wk", [PB, EB, SH])
    wv = din("wv", [PB, EB, SH])
    cwq = din("cwq", [PB, EB, SH]); cwk = din("cwk", [PB, EB, SH])
    cwv = din("cwv", [PB, EB, SH])
    w1 = din("w1", [PB, EB, H]); w2 = din("w2", [PB, EB, E])
    # packed params: [128, 8+64] f32 = bq|bk|cbq|cbk (2 each) then
    # b1|b2|g1|be1|g2|be2|g3|be3 (8 each)
    pk_f32 = din("pk_f32", [PB, 72], F32)
    # packed f32r col params: ones_c4 (4) | ones_128 (128)
    pk_dt = din("pk_dt", [PB, 132])
    # packed f32r row params: bvr(256)|cbvr(256)|ones_r128(128)|ones_r64(64)
    pk_row = din("pk_row", [1, 704])

    outT = nc.dram_tensor("outT", [PB, EB, TS], F32, kind="ExternalOutput").ap()

    cc1_in = [nc.dram_tensor(f"cc1_in{t}", [SH, 512], DT).ap() for t in range(2)]
    cc1_out = [nc.dram_tensor(f"cc1_out{t}", [E, 512], DT).ap() for t in range(2)]
    cc2_in = [nc.dram_tensor(f"cc2_in{t}", [SH, 512], DT).ap() for t in range(2)]
    cc2_all = nc.dram_tensor("cc2_all", [2 * E, 512], DT).ap()
    h1s_dram = nc.dram_tensor("h1s_dram", [E, TS], DT).ap()

    groups = [[0, 1, 2, 3], [4, 5, 6, 7]]

    with tile.TileContext(nc) as tc, \
         nc.allow_low_precision(reason="f32r rounding of matmul operands"):
        pid = nc.partition_id()
        off = (pid % GSZ) * TS            # token slice start for MLP
        tci = (pid % GSZ) // 2 * E        # AG2 chunk row base (0 or 1024)
        colo = (pid % 2) * TS             # column offset inside AG2 chunk

        # ---------- persistent pools (left stack bottom) ----------
        prm = tc.alloc_tile_pool(name="prm", bufs=1)
        pp = tc.alloc_tile_pool(name="pp", bufs=2, space="PSUM")
        sp = tc.alloc_tile_pool(name="sp", bufs=2, space="PSUM")
        op = tc.alloc_tile_pool(name="op", bufs=2, space="PSUM")
        epl = tc.alloc_tile_pool(name="epl", bufs=3)
        upl = tc.alloc_tile_pool(name="upl", bufs=3)
        rcp = tc.alloc_tile_pool(name="rcp", bufs=2)
        rdt = tc.alloc_tile_pool(name="rdt", bufs=2)
        sqp = tc.alloc_tile_pool(name="sqp", bufs=3)
        ltp = tc.alloc_tile_pool(name="ltp", bufs=3)
        lkp = tc.alloc_tile_pool(name="lkp", bufs=4)
        PERSIST = (prm, pp, sp, op, epl, upl, rcp, rdt, sqp, ltp, lkp)

        pkf = prm.tile([PB, 72], F32, tag="pkf", name="pkf")
        nc.gpsimd.dma_start(pkf[:], pk_f32[:])
        pkd = prm.tile([PB, 132], DT, tag="pkd", name="pkd")
        nc.gpsimd.dma_start(pkd[:], pk_dt[:])
        pkr = prm.tile([1, 704], DT, tag="pkr", name="pkr")
        nc.gpsimd.dma_start(pkr[:], pk_row[:])

        bq_s, bk_s, cbq_s, cbk_s = (pkf[:, 2 * i:2 * i + 2] for i in range(4))
        (b1_s, b2_s, g1_s, be1_s, g2_s, be2_s, g3_s, be3_s) = (
            pkf[:, 8 + 8 * i:16 + 8 * i] for i in range(8))
        o_c4 = pkd[:, 0:HPC]
        o_128 = pkd[:, HPC:HPC + PB]
        bvr_s = pkr[:, 0:SH]
        cbvr_s = pkr[:, SH:2 * SH]
        o_r128 = pkr[:, 2 * SH:2 * SH + PB]
        o_r64 = pkr[:, 2 * SH + PB:2 * SH + PB + HD]

        # ============ helpers ============
        def proj_qk(out_pool, out_tag, w_t, rhs_t, bias_sb):
            """2 blocks [128, 1024] = (W.T @ actT + bias)."""
            outs = [out_pool.tile([PB, P], DT, tag=out_tag, name=f"{out_tag}_{i}")
                    for i in range(2)]
            for mb in range(2):
                for tcx in range(2):
                    ps = pp.tile([PB, 512], F32, tag="pp", name="ps_proj")
                    for eb in range(EB):
                        nc.tensor.matmul(
                            ps[:], w_t[:, eb, mb * PB:(mb + 1) * PB],
                            rhs_t[:, eb, tcx * 512:(tcx + 1) * 512],
                            start=(eb == 0), stop=(eb == EB - 1))
                    nc.vector.tensor_scalar(
                        outs[mb][:, tcx * 512:(tcx + 1) * 512], ps[:],
                        bias_sb[:, mb:mb + 1], None, ADD)
            return outs

        def proj_v(v_pool, v_tag, act_t, wv_t, bvrow):
            """v tiles: 8 x [128, HPC, HD+1]; [:, h, :HD] = (act @ Wv + bv),
            [:, h, HD] = 1.0 (softmax-sum column)."""
            vts = []
            for jb in range(EB):
                ps = pp.tile([PB, SH], F32, tag="pp", name="ps_v")
                nc.tensor.matmul(ps[:], o_r128, bvrow, start=True, stop=False)
                for eb in range(EB):
                    nc.tensor.matmul(
                        ps[:], act_t[:, eb, jb * PB:(jb + 1) * PB],
                        wv_t[:, eb, :], start=False, stop=(eb == EB - 1))
                vt = v_pool.tile([PB, HPC, HD + 1], DT, tag=v_tag,
                                 name=f"{v_tag}_{jb}")
                for h in range(HPC):
                    nc.vector.tensor_copy(vt[:, h, 0:HD],
                                          ps[:, h * HD:(h + 1) * HD])
                nc.gpsimd.dma_start(vt[:, :, HD:HD + 1], o_c4[:, :, None])
                vts.append(vt)
            return vts

        def attention(qt, kt, vts, pa, tcx):
            """One 512-token chunk of 4-head attention; writes pa[hp][:, chunk]."""
            tsl = slice(tcx * 512, (tcx + 1) * 512)
            for hp in range(2):
                accs = [op.tile([HD + 1, 512], F32, tag="op", name=f"acc{i}")
                        for i in range(2)]
                for jb in range(EB):
                    s = sp.tile([PB, 1024], F32, tag="sp", name="s_scores")
                    for hi in range(2):
                        nc.tensor.matmul(
                            s[:, hi * 512:(hi + 1) * 512],
                            kt[hp][hi * HD:(hi + 1) * HD, jb * PB:(jb + 1) * PB],
                            qt[hp][hi * HD:(hi + 1) * HD, tsl],
                            start=True, stop=True)
                    ex = epl.tile([PB, 1024], DT, tag="ep", name="ex")
                    nc.scalar.activation(ex[:], s[:], Exp)
                    for hi in range(2):
                        nc.tensor.matmul(
                            accs[hi], vts[jb][:, 2 * hp + hi, :],
                            ex[:, hi * 512:(hi + 1) * 512],
                            start=(jb == 0), stop=(jb == EB - 1))
                for hi in range(2):
                    u = upl.tile([HD + 1, 512], F32, tag="up", name="u_out")
                    nc.vector.tensor_copy(u[:], accs[hi])
                    rec = rcp.tile([1, 512], F32, tag="rec", name="rec")
                    nc.vector.reciprocal(rec[:], u[HD:HD + 1, :])
                    rec_dt = rdt.tile([1, 512], DT, tag="recdt", name="rec_dt")
                    nc.vector.tensor_copy(rec_dt[:], rec[:])
                    bc = op.tile([HD, 512], F32, tag="op", name="bc")
                    nc.tensor.matmul(bc[:], o_r64, rec_dt[:], start=True, stop=True)
                    nc.vector.tensor_tensor(
                        pa[hp][hi * HD:(hi + 1) * HD, tsl], u[0:HD, :], bc[:], MULT)

        def ln_chunk(a_of, g_sb, b_sb, resid_fn, out_of, w):
            """LN over features (partitions) of a_of(eb) (8 x [128, w]) + affine
            + residual -> out_of(eb). Destroys a_of slices."""
            s1 = pp.tile([PB, w], F32, tag="pp", name="s1")
            s2 = pp.tile([PB, w], F32, tag="pp", name="s2")
            for eb in range(EB):
                sq = sqp.tile([PB, w], DT, tag="sq", name="sq")
                nc.vector.tensor_tensor(sq[:], a_of(eb), a_of(eb), MULT)
                nc.tensor.matmul(s1[:], o_128, a_of(eb),
                                 start=(eb == 0), stop=(eb == EB - 1))
                nc.tensor.matmul(s2[:], o_128, sq[:],
                                 start=(eb == 0), stop=(eb == EB - 1))
            mean = ltp.tile([PB, w], F32, tag="lt", name="mean")
            nc.scalar.mul(mean[:], s1[:], 1.0 / E)
            msq = ltp.tile([PB, w], F32, tag="lt", name="msq")
            nc.vector.tensor_tensor(msq[:], mean[:], mean[:], MULT)
            nc.vector.tensor_scalar(msq[:], msq[:], -1e-5, None, ADD)
            var = ltp.tile([PB, w], F32, tag="lt", name="var")
            nc.vector.scalar_tensor_tensor(var[:], s2[:], 1.0 / E, msq[:], MULT, SUB)
            sd = lkp.tile([PB, w], F32, tag="lk", name="sd")
            nc.scalar.activation(sd[:], var[:], Sqrt)
            rstd = lkp.tile([PB, w], F32, tag="lk", name="rstd")
            nc.vector.reciprocal(rstd[:], sd[:])
            nmr = lkp.tile([PB, w], F32, tag="lk", name="nmr")
            nc.vector.scalar_tensor_tensor(nmr[:], mean[:], -1.0, rstd[:], MULT, MULT)
            for eb in range(EB):
                a = a_of(eb)
                nc.vector.tensor_tensor(a, a, rstd[:], MULT)
                nc.vector.tensor_tensor(a, a, nmr[:], ADD)
                nc.vector.tensor_scalar(a, a, g_sb[:, eb:eb + 1],
                                        b_sb[:, eb:eb + 1], MULT, ADD)
                nc.vector.tensor_tensor(out_of(eb), a, resid_fn(eb), ADD)

        # ============ stage 1: big loads ============
        with nc.named_scope("loads"):
            xt_pool = tc.alloc_tile_pool(name="xt", bufs=1)
            xt = xt_pool.tile([PB, EB, P], DT, tag="xt", name="xt")
            nc.sync.dma_start(xt[:], xT[:])
            wsa_pool = tc.alloc_tile_pool(name="wsa", bufs=3)
            wq_t = wsa_pool.tile([PB, EB, SH], DT, tag="wsa", name="wq_t")
            nc.gpsimd.dma_start(wq_t[:], wq[:])
            wk_t = wsa_pool.tile([PB, EB, SH], DT, tag="wsa", name="wk_t")
            nc.gpsimd.dma_start(wk_t[:], wk[:])
            wv_t = wsa_pool.tile([PB, EB, SH], DT, tag="wsa", name="wv_t")
            nc.gpsimd.dma_start(wv_t[:], wv[:])

        # right stack: pa at the bottom
        pa_pool = tc.alloc_tile_pool(name="pa", bufs=2, side="right")
        pa = [pa_pool.tile([PB, P], DT, tag="pa", name=f"pa{i}") for i in range(2)]
        qk1_pool = tc.alloc_tile_pool(name="qk1", bufs=4, side="right")
        v1_pool = tc.alloc_tile_pool(name="v1", bufs=EB, side="right")

        # ============ stage 2: SA projections ============
        with nc.named_scope("sa_proj"):
            qt = proj_qk(qk1_pool, "qk1", wq_t, xt, bq_s)
            kt = proj_qk(qk1_pool, "qk1", wk_t, xt, bk_s)
            vts = proj_v(v1_pool, "v1", xt, wv_t, bvr_s)
        wsa_pool.release()

        # ctx load (DMA overlaps SA attention)
        ctx_pool = tc.alloc_tile_pool(name="ctx", bufs=1)
        ctx = ctx_pool.tile([PB, EB, C], DT, tag="ctx", name="ctx")
        nc.sync.dma_start(ctx[:], ctxT[:])

        # ============ stage 3: SA attention + chunked AllGather ============
        with nc.named_scope("sa_attn"):
            for tcx in range(2):
                attention(qt, kt, vts, pa, tcx)
                tsl = slice(tcx * 512, (tcx + 1) * 512)
                for mb in range(2):
                    nc.sync.dma_start(cc1_in[tcx][mb * PB:(mb + 1) * PB, :],
                                      pa[mb][:, tsl])
                nc.gpsimd.collective_compute(
                    "AllGather", mybir.AluOpType.bypass, replica_groups=groups,
                    ins=[cc1_in[tcx][:]], outs=[cc1_out[tcx][:]])
        v1_pool.release()
        qk1_pool.release()

        # ============ stage 4: CA weight loads + k/v projections ============
        qk2_pool = tc.alloc_tile_pool(name="qk2", bufs=4, side="right")
        v2_pool = tc.alloc_tile_pool(name="v2", bufs=EB, side="right")
        wca_pool = tc.alloc_tile_pool(name="wca", bufs=3, side="right")
        with nc.named_scope("ca_kv"):
            cwk_t = wca_pool.tile([PB, EB, SH], DT, tag="wca", name="cwk_t")
            nc.gpsimd.dma_start(cwk_t[:], cwk[:])
            cwv_t = wca_pool.tile([PB, EB, SH], DT, tag="wca", name="cwv_t")
            nc.gpsimd.dma_start(cwv_t[:], cwv[:])
            cwq_t = wca_pool.tile([PB, EB, SH], DT, tag="wca", name="cwq_t")
            nc.gpsimd.dma_start(cwq_t[:], cwq[:])
            k2t = proj_qk(qk2_pool, "qk2", cwk_t, ctx, cbk_s)
            v2ts = proj_v(v2_pool, "v2", ctx, cwv_t, cbvr_s)
        ctx_pool.release()

        # ============ stage 5: LN1 + residual -> h1 ============
        h1_pool = tc.alloc_tile_pool(name="h1", bufs=1, side="right")
        h1 = h1_pool.tile([PB, EB, P], DT, tag="h1", name="h1")
        a1_pool = tc.alloc_tile_pool(name="a1", bufs=1, side="right")
        with nc.named_scope("ln1"):
            for tcx in range(2):
                a1 = a1_pool.tile([PB, EB, 512], DT, tag="a1", name="a1")
                nc.sync.dma_start(
                    a1[:], cc1_out[tcx].rearrange("(e p) n -> p e n", p=PB))
                tsl = slice(tcx * 512, (tcx + 1) * 512)
                ln_chunk(lambda eb: a1[:, eb, :], g1_s, be1_s,
                         lambda eb: xt[:, eb, tsl],
                         lambda eb: h1[:, eb, tsl], 512)
            # stash my token slice of h1 for the LN2 residual
            nc.sync.dma_start(
                h1s_dram.rearrange("(e p) n -> p e n", p=PB),
                h1[:, :, ds(off, TS)])
        a1_pool.release()
        xt_pool.release()

        # ============ stage 6: CA q projection + attention ============
        with nc.named_scope("ca_q"):
            qt2 = proj_qk(qk2_pool, "qk2", cwq_t, h1, cbq_s)
        h1_pool.release()
        wca_pool.release()

        # left stack: tail pools (w1/w2 DMAs overlap CA attention)
        out_pool = tc.alloc_tile_pool(name="outp", bufs=1)
        y_pool = tc.alloc_tile_pool(name="yp", bufs=1)
        h2_pool = tc.alloc_tile_pool(name="h2", bufs=1)
        w2_pool = tc.alloc_tile_pool(name="w2p", bufs=1)
        m1_pool = tc.alloc_tile_pool(name="m1", bufs=1)
        w1_pool = tc.alloc_tile_pool(name="w1p", bufs=1)
        rtp = tc.alloc_tile_pool(name="rtp", bufs=1)
        a2_pool = tc.alloc_tile_pool(name="a2", bufs=1)
        w1_t = w1_pool.tile([PB, EB, H], DT, tag="w1", name="w1_t")
        nc.gpsimd.dma_start(w1_t[:], w1[:])
        w2_t = w2_pool.tile([PB, EB, E], DT, tag="w2", name="w2_t")
        nc.gpsimd.dma_start(w2_t[:], w2[:])

        with nc.named_scope("ca_attn"):
            for tcx in range(2):
                attention(qt2, k2t, v2ts, pa, tcx)
                tsl = slice(tcx * 512, (tcx + 1) * 512)
                for mb in range(2):
                    nc.sync.dma_start(cc2_in[tcx][mb * PB:(mb + 1) * PB, :],
                                      pa[mb][:, tsl])
                nc.gpsimd.collective_compute(
                    "AllGather", mybir.AluOpType.bypass, replica_groups=groups,
                    ins=[cc2_in[tcx][:]],
                    outs=[cc2_all[tcx * E:(tcx + 1) * E, :]])
        v2_pool.release()
        qk2_pool.release()
        pa_pool.release()

        # ============ stage 7: LN2 + residual on my token slice -> h2 ============
        with nc.named_scope("ln2_mlp"):
            h2 = h2_pool.tile([PB, EB, TS], DT, tag="h2", name="h2")
            a2 = a2_pool.tile([PB, EB, TS], DT, tag="a2", name="a2")
            nc.sync.dma_start(
                a2[:],
                cc2_all[ds(tci, E), ds(colo, TS)].rearrange(
                    "(e p) n -> p e n", p=PB))
            rt = rtp.tile([PB, EB, TS], DT, tag="rt", name="rt")
            nc.sync.dma_start(rt[:], h1s_dram.rearrange("(e p) n -> p e n", p=PB))
            ln_chunk(lambda eb: a2[:, eb, :], g2_s, be2_s,
                     lambda eb: rt[:, eb, :],
                     lambda eb: h2[:, eb, :], TS)
            a2_pool.release()
            rtp.release()

            # ============ stage 8: MLP ============
            m1 = m1_pool.tile([PB, EB, TS], DT, tag="m1", name="m1")
            for hb in range(EB):
                ps = pp.tile([PB, TS], F32, tag="pp", name="ps_m1")
                for eb in range(EB):
                    nc.tensor.matmul(ps[:], w1_t[:, eb, hb * PB:(hb + 1) * PB],
                                     h2[:, eb, :], start=(eb == 0),
                                     stop=(eb == EB - 1))
                nc.vector.tensor_scalar(m1[:, hb, :], ps[:], b1_s[:, hb:hb + 1],
                                        0.0, ADD, MAX)
            w1_pool.release()

            y = y_pool.tile([PB, EB, TS], DT, tag="y", name="y")
            for eb in range(EB):
                ps = pp.tile([PB, TS], F32, tag="pp", name="ps_y")
                for hb in range(EB):
                    nc.tensor.matmul(ps[:], w2_t[:, hb, eb * PB:(eb + 1) * PB],
                                     m1[:, hb, :], start=(hb == 0),
                                     stop=(hb == EB - 1))
                nc.vector.tensor_scalar(y[:, eb, :], ps[:], b2_s[:, eb:eb + 1],
                                        None, ADD)
            m1_pool.release()
            w2_pool.release()

        # ============ stage 9: LN3 + residual -> output ============
        with nc.named_scope("ln3"):
            ot = out_pool.tile([PB, EB, TS], F32, tag="ot", name="ot")
            ln_chunk(lambda eb: y[:, eb, :], g3_s, be3_s,
                     lambda eb: h2[:, eb, :],
                     lambda eb: ot[:, eb, :], TS)
            nc.sync.dma_start(outT[:], ot[:])
        h2_pool.release()
        y_pool.release()
        out_pool.release()
        for _pl in reversed(PERSIST):
            _pl.release()

    nc.compile()
    return nc


def _blk(M):
    """[1024, w] -> [128, 8, w] partition-blocked contiguous."""
    return np.ascontiguousarray(M.reshape(EB, PB, -1).transpose(1, 0, 2))


def _host_prep(inputs):
    f = {k: np.ascontiguousarray(np.asarray(v, dtype=np.float32))
         for k, v in inputs.items()}
    perm = np.empty(E, dtype=np.int64)
    for h in range(NH):
        for d in range(HD):
            perm[h * HD + d] = d * NH + h
    inv = np.argsort(perm)
    s = np.float32(1.0 / np.sqrt(HD))

    sa_Wq = f["sa_Wq"][perm][:, perm] * s
    sa_bq = f["sa_bq"][perm] * s
    sa_Wk = f["sa_Wk"][perm][:, perm]; sa_bk = f["sa_bk"][perm]
    sa_Wv = f["sa_Wv"][perm][:, perm]; sa_bv = f["sa_bv"][perm]
    ca_Wq = f["ca_Wq"][perm][:, perm] * s
    ca_bq = f["ca_bq"][perm] * s
    ca_Wk = f["ca_Wk"][:, perm]; ca_bk = f["ca_bk"][perm]
    ca_Wv = f["ca_Wv"][:, perm]; ca_bv = f["ca_bv"][perm]
    W1 = _blk(f["mlp_W1"][perm, :])
    W2 = _blk(f["mlp_W2"][:, perm])
    b2p = f["mlp_b2"][perm]

    def pcol(v):  # [X] -> [128, X//128] block-major per-partition layout
        return np.ascontiguousarray(v.reshape(-1, PB).T)

    in_maps = []
    for c in range(NCORES):
        b, r = c // GSZ, c % GSZ
        sl = slice(r * SH, (r + 1) * SH)
        pkf = np.concatenate(
            [pcol(sa_bq[sl]), pcol(sa_bk[sl]), pcol(ca_bq[sl]), pcol(ca_bk[sl]),
             pcol(f["mlp_b1"]), pcol(b2p),
             pcol(f["ln1_g"][perm]), pcol(f["ln1_b"][perm]),
             pcol(f["ln2_g"][perm]), pcol(f["ln2_b"][perm]),
             pcol(f["ln3_g"][perm]), pcol(f["ln3_b"][perm])], axis=1)
        pkd = np.concatenate(
            [np.ones((PB, HPC), np.float32), np.ones((PB, PB), np.float32)],
            axis=1)
        pkrow = np.concatenate(
            [sa_bv[sl], ca_bv[sl], np.ones(PB, np.float32),
             np.ones(HD, np.float32)])[None, :]
        m = {
            "xT": _blk(np.ascontiguousarray(f["x"][b][:, perm].T)),
            "ctxT": _blk(np.ascontiguousarray(f["context"][b].T)),
            "wq": _blk(sa_Wq[:, sl]), "wk": _blk(sa_Wk[:, sl]),
            "wv": _blk(sa_Wv[:, sl]),
            "cwq": _blk(ca_Wq[:, sl]), "cwk": _blk(ca_Wk[:, sl]),
            "cwv": _blk(ca_Wv[:, sl]),
            "w1": W1, "w2": W2,
            "pk_f32": np.ascontiguousarray(pkf),
            "pk_dt": np.ascontiguousarray(pkd),
            "pk_row": np.ascontiguousarray(pkrow),
        }
        in_maps.append(m)
    return in_maps, inv


def kernel(**inputs) -> np.ndarray:
    global _COMPILED
    if _COMPILED is None:
        _COMPILED = _build()
    nc = _COMPILED
    in_maps, inv = _host_prep(inputs)
    res = run_bass_kernel_spmd(nc, in_maps, list(range(NCORES)), trace=TRACE)
    kernel.last_results = res
    out = np.empty((B, P, E), np.float32)
    for c in range(NCORES):
        b, r = c // GSZ, c % GSZ
        o = res.results[c]["outT"]          # [128, 8, 256]
        out[b, r * TS:(r + 1) * TS, :] = o.transpose(1, 0, 2).reshape(E, TS).T
    return np.ascontiguousarray(out[:, :, inv])



# revision 9
# speedup vs baseline: 1.0457x; 1.0457x over previous
"""Trainium2 Bass kernel for nn_DecoderLayer (dense transformer decoder layer).

Full inputs in, full output out. Internally sharded across 8 NeuronCores:
  core c (0..7): batch b = c//4, group rank r = c%4
  - features permuted so heads are contiguous: perm[h*64+d] = d*16+h
  - self/cross attention: head-parallel, 4 heads per core (256 permuted cols)
  - AllGather of attention outputs over each 4-core group
  - layernorm computed redundantly per group (transposed layout, partition
    reductions via ones-matmuls)
  - MLP + final LN: token-parallel, 256 tokens per core, assembled on host

All activations live in transposed layout [feature, token]; weights are
pre-permuted/sliced/scaled on the host and pre-blocked to [128, 8, w] so each
load is a single DMA with 8-32KB contiguous per-partition lines. Matmuls run
in float32r (full rate on TRN2 for free dim >= 256).
"""
import sys

if "/opt/trn_rl_repo" not in sys.path:
    sys.path.insert(0, "/opt/trn_rl_repo")

import numpy as np

import concourse.bass as bass
import concourse.mybir as mybir
import concourse.tile as tile
from concourse import bacc
from concourse.bass import ds
from concourse.bass_utils import run_bass_kernel_spmd

# ---- problem constants (hardcoded per contract) ----
E = 1024          # embed dim
H = 1024          # hidden dim
NH = 16           # heads
HD = 64           # head dim
B = 2             # batch
P = 1024          # problem (query) length
C = 1024          # context length
NCORES = 8
GSZ = 4           # cores per batch group
HPC = NH // GSZ   # heads per core = 4
SH = HPC * HD     # per-core head shard width = 256
TS = P // GSZ     # per-core token slice for MLP = 256
PB = 128          # partitions
EB = E // PB      # 8 feature blocks

DT = mybir.dt.bfloat16
F32 = mybir.dt.float32
Exp = mybir.ActivationFunctionType.Exp
Sqrt = mybir.ActivationFunctionType.Sqrt
ADD = mybir.AluOpType.add
MULT = mybir.AluOpType.mult
SUB = mybir.AluOpType.subtract
MAX = mybir.AluOpType.max

_COMPILED = None
TRACE = False      # test harness may flip this


def _build():
    nc = bacc.Bacc(trn_type="TRN2", num_devices=NCORES,
                   target_bir_lowering=False, debug=False)

    def din(name, shape, dt=DT):
        return nc.dram_tensor(name, shape, dt, kind="ExternalInput").ap()

    # big inputs, host pre-blocked to [128, EB, w]
    xT = din("xT", [PB, EB, P])
    ctxT = din("ctxT", [PB, EB, C])
    wq = din("wq", [PB, EB, SH]); wk = din("wk", [PB, EB, SH])
    wv = din("wv", [PB, EB, SH])
    cwq = din("cwq", [PB, EB, SH]); cwk = din("cwk", [PB, EB, SH])
    cwv = din("cwv", [PB, EB, SH])
    w1 = din("w1", [PB, EB, H]); w2 = din("w2", [PB, EB, E])
    # packed params: [128, 8+64] f32 = bq|bk|cbq|cbk (2 each) then
    # b1|b2|g1|be1|g2|be2|g3|be3 (8 each)
    pk_f32 = din("pk_f32", [PB, 72], F32)
    # packed f32r col params: ones_c4 (4) | ones_128 (128)
    pk_dt = din("pk_dt", [PB, 132])
    # packed f32r row params: bvr(256)|cbvr(256)|ones_r128(128)|ones_r64(64)
    pk_row = din("pk_row", [1, 704])

    outT = nc.dram_tensor("outT", [PB, EB, TS], F32, kind="ExternalOutput").ap()

    cc1_in = [nc.dram_tensor(f"cc1_in{t}", [SH, 512], DT).ap() for t in range(2)]
    cc1_out = [nc.dram_tensor(f"cc1_out{t}", [E, 512], DT).ap() for t in range(2)]
    cc2_in = [nc.dram_tensor(f"cc2_in{t}", [SH, 512], DT).ap() for t in range(2)]
    cc2_all = nc.dram_tensor("cc2_all", [2 * E, 512], DT).ap()
    h1s_dram = nc.dram_tensor("h1s_dram", [E, TS], DT).ap()

    groups = [[0, 1, 2, 3], [4, 5, 6, 7]]

    with tile.TileContext(nc) as tc, \
         nc.allow_low_precision(reason="f32r rounding of matmul operands"):
        pid = nc.partition_id()
        off = (pid % GSZ) * TS            # token slice start for MLP
        tci = (pid % GSZ) // 2 * E        # AG2 chunk row base (0 or 1024)
        colo = (pid % 2) * TS             # column offset inside AG2 chunk

        # ---------- persistent pools (left stack bottom) ----------
        prm = tc.alloc_tile_pool(name="prm", bufs=1)
        pp = tc.alloc_tile_pool(name="pp", bufs=2, space="PSUM")
        sp = tc.alloc_tile_pool(name="sp", bufs=2, space="PSUM")
        op = tc.alloc_tile_pool(name="op", bufs=2, space="PSUM")
        epl = tc.alloc_tile_pool(name="epl", bufs=3)
        upl = tc.alloc_tile_pool(name="upl", bufs=5)
        rcp = tc.alloc_tile_pool(name="rcp", bufs=2)
        rdt = tc.alloc_tile_pool(name="rdt", bufs=2)
        sqp = tc.alloc_tile_pool(name="sqp", bufs=9)
        ltp = tc.alloc_tile_pool(name="ltp", bufs=3)
        lkp = tc.alloc_tile_pool(name="lkp", bufs=4)
        PERSIST = (prm, pp, sp, op, epl, upl, rcp, rdt, sqp, ltp, lkp)

        pkf = prm.tile([PB, 72], F32, tag="pkf", name="pkf")
        nc.gpsimd.dma_start(pkf[:], pk_f32[:])
        pkd = prm.tile([PB, 132], DT, tag="pkd", name="pkd")
        nc.gpsimd.dma_start(pkd[:], pk_dt[:])
        pkr = prm.tile([1, 704], DT, tag="pkr", name="pkr")
        nc.gpsimd.dma_start(pkr[:], pk_row[:])

        bq_s, bk_s, cbq_s, cbk_s = (pkf[:, 2 * i:2 * i + 2] for i in range(4))
        (b1_s, b2_s, g1_s, be1_s, g2_s, be2_s, g3_s, be3_s) = (
            pkf[:, 8 + 8 * i:16 + 8 * i] for i in range(8))
        o_c4 = pkd[:, 0:HPC]
        o_128 = pkd[:, HPC:HPC + PB]
        bvr_s = pkr[:, 0:SH]
        cbvr_s = pkr[:, SH:2 * SH]
        o_r128 = pkr[:, 2 * SH:2 * SH + PB]
        o_r64 = pkr[:, 2 * SH + PB:2 * SH + PB + HD]

        # ============ helpers ============
        def proj_qk(out_pool, out_tag, w_t, rhs_t, bias_sb):
            """2 blocks [128, 1024] = (W.T @ actT + bias)."""
            outs = [out_pool.tile([PB, P], DT, tag=out_tag, name=f"{out_tag}_{i}")
                    for i in range(2)]
            for mb in range(2):
                for tcx in range(2):
                    ps = pp.tile([PB, 512], F32, tag="pp", name="ps_proj")
                    for eb in range(EB):
                        nc.tensor.matmul(
                            ps[:], w_t[:, eb, mb * PB:(mb + 1) * PB],
                            rhs_t[:, eb, tcx * 512:(tcx + 1) * 512],
                            start=(eb == 0), stop=(eb == EB - 1))
                    nc.vector.tensor_scalar(
                        outs[mb][:, tcx * 512:(tcx + 1) * 512], ps[:],
                        bias_sb[:, mb:mb + 1], None, ADD)
            return outs

        def proj_v(v_pool, v_tag, act_t, wv_t, bvrow):
            """v tiles: 8 x [128, HPC, HD+1]; [:, h, :HD] = (act @ Wv + bv),
            [:, h, HD] = 1.0 (softmax-sum column)."""
            vts = []
            for jb in range(EB):
                ps = pp.tile([PB, SH], F32, tag="pp", name="ps_v")
                nc.tensor.matmul(ps[:], o_r128, bvrow, start=True, stop=False)
                for eb in range(EB):
                    nc.tensor.matmul(
                        ps[:], act_t[:, eb, jb * PB:(jb + 1) * PB],
                        wv_t[:, eb, :], start=False, stop=(eb == EB - 1))
                vt = v_pool.tile([PB, HPC, HD + 1], DT, tag=v_tag,
                                 name=f"{v_tag}_{jb}")
                for h in range(HPC):
                    nc.vector.tensor_copy(vt[:, h, 0:HD],
                                          ps[:, h * HD:(h + 1) * HD])
                nc.gpsimd.dma_start(vt[:, :, HD:HD + 1], o_c4[:, :, None])
                vts.append(vt)
            return vts

        def attention(qt, kt, vts, pa, tcx):
            """One 512-token chunk of 4-head attention; writes pa[hp][:, chunk].
            Software-pipelined: scores/exp for jb+1 are emitted before the
            accumulate matmuls of jb so the PE never waits on the exp."""
            tsl = slice(tcx * 512, (tcx + 1) * 512)
            us = []
            for hp in range(2):
                accs = [op.tile([HD + 1, 512], F32, tag="op", name=f"acc{i}")
                        for i in range(2)]
                exs = {}
                for jb in range(EB + 1):
                    if jb < EB:
                        s = sp.tile([PB, 1024], F32, tag="sp", name="s_scores")
                        for hi in range(2):
                            nc.tensor.matmul(
                                s[:, hi * 512:(hi + 1) * 512],
                                kt[hp][hi * HD:(hi + 1) * HD, jb * PB:(jb + 1) * PB],
                                qt[hp][hi * HD:(hi + 1) * HD, tsl],
                                start=True, stop=True)
                        ex = epl.tile([PB, 1024], DT, tag="ep", name="ex")
                        nc.scalar.activation(ex[:], s[:], Exp)
                        exs[jb] = ex
                    if jb > 0:
                        pj = jb - 1
                        ex = exs.pop(pj)
                        for hi in range(2):
                            nc.tensor.matmul(
                                accs[hi], vts[pj][:, 2 * hp + hi, :],
                                ex[:, hi * 512:(hi + 1) * 512],
                                start=(pj == 0), stop=(pj == EB - 1))
                for hi in range(2):
                    u = upl.tile([HD + 1, 512], F32, tag="up", name="u_out")
                    nc.scalar.copy(u[:], accs[hi])
                    rec_dt = rdt.tile([1, 512], DT, tag="recdt", name="rec_dt")
                    nc.vector.reciprocal(rec_dt[:], u[HD:HD + 1, :])
                    bc = pp.tile([HD, 512], F32, tag="pp", name="bc")
                    nc.tensor.matmul(bc[:], o_r64, rec_dt[:],
                                     start=True, stop=True)
                    nc.vector.tensor_tensor(
                        pa[hp][hi * HD:(hi + 1) * HD, tsl], u[0:HD, :],
                        bc[:], MULT)

        def ln_chunk(a_of, g_sb, b_sb, resid_fn, out_of, w):
            """LN over features (partitions) of a_of(eb) (8 x [128, w]) + affine
            + residual -> out_of(eb). Destroys a_of slices."""
            s1 = pp.tile([PB, w], F32, tag="pp", name="s1")
            s2 = pp.tile([PB, w], F32, tag="pp", name="s2")
            sqs = []
            for eb in range(EB):
                sq = sqp.tile([PB, w], DT, tag="sq", name="sq")
                nc.vector.tensor_tensor(sq[:], a_of(eb), a_of(eb), MULT)
                sqs.append(sq)
                nc.tensor.matmul(s1[:], o_128, a_of(eb),
                                 start=(eb == 0), stop=(eb == EB - 1))
            for eb in range(EB):
                nc.tensor.matmul(s2[:], o_128, sqs[eb][:],
                                 start=(eb == 0), stop=(eb == EB - 1))
            mean = ltp.tile([PB, w], F32, tag="lt", name="mean")
            nc.scalar.mul(mean[:], s1[:], 1.0 / E)
            msq = ltp.tile([PB, w], F32, tag="lt", name="msq")
            nc.vector.tensor_tensor(msq[:], mean[:], mean[:], MULT)
            nc.vector.tensor_scalar(msq[:], msq[:], -1e-5, None, ADD)
            var = ltp.tile([PB, w], F32, tag="lt", name="var")
            nc.vector.scalar_tensor_tensor(var[:], s2[:], 1.0 / E, msq[:], MULT, SUB)
            rstd = lkp.tile([PB, w], F32, tag="lk", name="rstd")
            nc.scalar.activation(rstd[:], var[:],
                                 mybir.ActivationFunctionType.Abs_reciprocal_sqrt)
            nmr = lkp.tile([PB, w], F32, tag="lk", name="nmr")
            nc.vector.scalar_tensor_tensor(nmr[:], mean[:], -1.0, rstd[:], MULT, MULT)
            for eb in range(EB):
                a = a_of(eb)
                nc.vector.tensor_tensor(a, a, rstd[:], MULT)
                nc.vector.tensor_tensor(a, a, nmr[:], ADD)
                nc.vector.tensor_scalar(a, a, g_sb[:, eb:eb + 1],
                                        b_sb[:, eb:eb + 1], MULT, ADD)
                nc.vector.tensor_tensor(out_of(eb), a, resid_fn(eb), ADD)

        # ============ stage 1: big loads ============
        with nc.named_scope("loads"):
            xt_pool = tc.alloc_tile_pool(name="xt", bufs=1)
            xt = xt_pool.tile([PB, EB, P], DT, tag="xt", name="xt")
            for eb in range(EB):
                nc.sync.dma_start(xt[:, eb, :], xT[:, eb, :])
            wsa_pool = tc.alloc_tile_pool(name="wsa", bufs=3)
            wq_t = wsa_pool.tile([PB, EB, SH], DT, tag="wsa", name="wq_t")
            nc.gpsimd.dma_start(wq_t[:], wq[:])
            wk_t = wsa_pool.tile([PB, EB, SH], DT, tag="wsa", name="wk_t")
            nc.gpsimd.dma_start(wk_t[:], wk[:])
            wv_t = wsa_pool.tile([PB, EB, SH], DT, tag="wsa", name="wv_t")
            nc.gpsimd.dma_start(wv_t[:], wv[:])

        # right stack: pa at the bottom
        pa_pool = tc.alloc_tile_pool(name="pa", bufs=2, side="right")
        pa = [pa_pool.tile([PB, P], DT, tag="pa", name=f"pa{i}") for i in range(2)]
        qk1_pool = tc.alloc_tile_pool(name="qk1", bufs=4, side="right")
        v1_pool = tc.alloc_tile_pool(name="v1", bufs=EB, side="right")

        # ============ stage 2: SA projections ============
        with nc.named_scope("sa_proj"):
            qt = proj_qk(qk1_pool, "qk1", wq_t, xt, bq_s)
            kt = proj_qk(qk1_pool, "qk1", wk_t, xt, bk_s)
            vts = proj_v(v1_pool, "v1", xt, wv_t, bvr_s)
        wsa_pool.release()

        # ctx load (DMA overlaps SA attention)
        ctx_pool = tc.alloc_tile_pool(name="ctx", bufs=1)
        ctx = ctx_pool.tile([PB, EB, C], DT, tag="ctx", name="ctx")
        for cb in range(2):
            nc.sync.dma_start(ctx[:, 4 * cb:4 * (cb + 1), :],
                              ctxT[:, 4 * cb:4 * (cb + 1), :])

        # ============ stage 3: SA attention + chunked AllGather ============
        with nc.named_scope("sa_attn"):
            for tcx in range(2):
                attention(qt, kt, vts, pa, tcx)
                tsl = slice(tcx * 512, (tcx + 1) * 512)
                for mb in range(2):
                    nc.sync.dma_start(cc1_in[tcx][mb * PB:(mb + 1) * PB, :],
                                      pa[mb][:, tsl])
                nc.gpsimd.collective_compute(
                    "AllGather", mybir.AluOpType.bypass, replica_groups=groups,
                    ins=[cc1_in[tcx][:]], outs=[cc1_out[tcx][:]])
        v1_pool.release()
        qk1_pool.release()

        # ============ stage 4: CA weight loads + k/v projections ============
        qk2_pool = tc.alloc_tile_pool(name="qk2", bufs=4, side="right")
        v2_pool = tc.alloc_tile_pool(name="v2", bufs=EB, side="right")
        wca_pool = tc.alloc_tile_pool(name="wca", bufs=3, side="right")
        with nc.named_scope("ca_kv"):
            cwk_t = wca_pool.tile([PB, EB, SH], DT, tag="wca", name="cwk_t")
            nc.gpsimd.dma_start(cwk_t[:], cwk[:])
            cwv_t = wca_pool.tile([PB, EB, SH], DT, tag="wca", name="cwv_t")
            nc.gpsimd.dma_start(cwv_t[:], cwv[:])
            cwq_t = wca_pool.tile([PB, EB, SH], DT, tag="wca", name="cwq_t")
            nc.gpsimd.dma_start(cwq_t[:], cwq[:])
            k2t = proj_qk(qk2_pool, "qk2", cwk_t, ctx, cbk_s)
            v2ts = proj_v(v2_pool, "v2", ctx, cwv_t, cbvr_s)
        ctx_pool.release()

        # ============ stage 5: LN1 + residual -> h1 ============
        h1_pool = tc.alloc_tile_pool(name="h1", bufs=1, side="right")
        h1 = h1_pool.tile([PB, EB, P], DT, tag="h1", name="h1")
        a1_pool = tc.alloc_tile_pool(name="a1", bufs=1, side="right")
        with nc.named_scope("ln1"):
            for tcx in range(2):
                a1 = a1_pool.tile([PB, EB, 512], DT, tag="a1", name="a1")
                nc.sync.dma_start(
                    a1[:], cc1_out[tcx].rearrange("(e p) n -> p e n", p=PB))
                tsl = slice(tcx * 512, (tcx + 1) * 512)
                ln_chunk(lambda eb: a1[:, eb, :], g1_s, be1_s,
                         lambda eb: xt[:, eb, tsl],
                         lambda eb: h1[:, eb, tsl], 512)
            # stash my token slice of h1 for the LN2 residual
            nc.sync.dma_start(
                h1s_dram.rearrange("(e p) n -> p e n", p=PB),
                h1[:, :, ds(off, TS)])
        a1_pool.release()
        xt_pool.release()

        # ============ stage 6: CA q projection + attention ============
        with nc.named_scope("ca_q"):
            qt2 = proj_qk(qk2_pool, "qk2", cwq_t, h1, cbq_s)
        h1_pool.release()
        wca_pool.release()

        # left stack: tail pools (w1/w2 DMAs overlap CA attention)
        out_pool = tc.alloc_tile_pool(name="outp", bufs=1)
        y_pool = tc.alloc_tile_pool(name="yp", bufs=1)
        h2_pool = tc.alloc_tile_pool(name="h2", bufs=1)
        w2_pool = tc.alloc_tile_pool(name="w2p", bufs=1)
        m1_pool = tc.alloc_tile_pool(name="m1", bufs=1)
        w1_pool = tc.alloc_tile_pool(name="w1p", bufs=1)
        rtp = tc.alloc_tile_pool(name="rtp", bufs=1)
        a2_pool = tc.alloc_tile_pool(name="a2", bufs=1)
        w1_t = w1_pool.tile([PB, EB, H], DT, tag="w1", name="w1_t")
        nc.gpsimd.dma_start(w1_t[:], w1[:])
        w2_t = w2_pool.tile([PB, EB, E], DT, tag="w2", name="w2_t")
        nc.gpsimd.dma_start(w2_t[:], w2[:])

        with nc.named_scope("ca_attn"):
            for tcx in range(2):
                attention(qt2, k2t, v2ts, pa, tcx)
                tsl = slice(tcx * 512, (tcx + 1) * 512)
                for mb in range(2):
                    nc.sync.dma_start(cc2_in[tcx][mb * PB:(mb + 1) * PB, :],
                                      pa[mb][:, tsl])
                nc.gpsimd.collective_compute(
                    "AllGather", mybir.AluOpType.bypass, replica_groups=groups,
                    ins=[cc2_in[tcx][:]],
                    outs=[cc2_all[tcx * E:(tcx + 1) * E, :]])
        v2_pool.release()
        qk2_pool.release()
        pa_pool.release()

        # ============ stage 7: LN2 + residual on my token slice -> h2 ============
        with nc.named_scope("ln2_mlp"):
            h2 = h2_pool.tile([PB, EB, TS], DT, tag="h2", name="h2")
            a2 = a2_pool.tile([PB, EB, TS], DT, tag="a2", name="a2")
            nc.sync.dma_start(
                a2[:],
                cc2_all[ds(tci, E), ds(colo, TS)].rearrange(
                    "(e p) n -> p e n", p=PB))
            rt = rtp.tile([PB, EB, TS], DT, tag="rt", name="rt")
            nc.sync.dma_start(rt[:], h1s_dram.rearrange("(e p) n -> p e n", p=PB))
            ln_chunk(lambda eb: a2[:, eb, :], g2_s, be2_s,
                     lambda eb: rt[:, eb, :],
                     lambda eb: h2[:, eb, :], TS)
            a2_pool.release()
            rtp.release()

            # ============ stage 8: MLP ============
            m1 = m1_pool.tile([PB, EB, TS], DT, tag="m1", name="m1")
            for hb in range(EB):
                ps = pp.tile([PB, TS], F32, tag="pp", name="ps_m1")
                for eb in range(EB):
                    nc.tensor.matmul(ps[:], w1_t[:, eb, hb * PB:(hb + 1) * PB],
                                     h2[:, eb, :], start=(eb == 0),
                                     stop=(eb == EB - 1))
                nc.vector.tensor_scalar(m1[:, hb, :], ps[:], b1_s[:, hb:hb + 1],
                                        0.0, ADD, MAX)
            w1_pool.release()

            y = y_pool.tile([PB, EB, TS], DT, tag="y", name="y")
            for eb in range(EB):
                ps = pp.tile([PB, TS], F32, tag="pp", name="ps_y")
                for hb in range(EB):
                    nc.tensor.matmul(ps[:], w2_t[:, hb, eb * PB:(eb + 1) * PB],
                                     m1[:, hb, :], start=(hb == 0),
                                     stop=(hb == EB - 1))
                nc.vector.tensor_scalar(y[:, eb, :], ps[:], b2_s[:, eb:eb + 1],
                                        None, ADD)
            m1_pool.release()
            w2_pool.release()

        # ============ stage 9: LN3 + residual -> output ============
        with nc.named_scope("ln3"):
            ot = out_pool.tile([PB, EB, TS], F32, tag="ot", name="ot")
            ln_chunk(lambda eb: y[:, eb, :], g3_s, be3_s,
                     lambda eb: h2[:, eb, :],
                     lambda eb: ot[:, eb, :], TS)
            nc.sync.dma_start(outT[:], ot[:])
        h2_pool.release()
        y_pool.release()
        out_pool.release()
        for _pl in reversed(PERSIST):
            _pl.release()

    nc.compile()
    return nc


def _blk(M):
    """[1024, w] -> [128, 8, w] partition-blocked contiguous."""
    return np.ascontiguousarray(M.reshape(EB, PB, -1).transpose(1, 0, 2))


def _host_prep(inputs):
    import ml_dtypes
    bf16 = ml_dtypes.bfloat16
    f = {k: np.ascontiguousarray(np.asarray(v, dtype=np.float32))
         for k, v in inputs.items()}
    perm = np.empty(E, dtype=np.int64)
    for h in range(NH):
        for d in range(HD):
            perm[h * HD + d] = d * NH + h
    inv = np.argsort(perm)
    s = np.float32(1.0 / np.sqrt(HD))

    sa_Wq = f["sa_Wq"][perm][:, perm] * s
    sa_bq = f["sa_bq"][perm] * s
    sa_Wk = f["sa_Wk"][perm][:, perm]; sa_bk = f["sa_bk"][perm]
    sa_Wv = f["sa_Wv"][perm][:, perm]; sa_bv = f["sa_bv"][perm]
    ca_Wq = f["ca_Wq"][perm][:, perm] * s
    ca_bq = f["ca_bq"][perm] * s
    ca_Wk = f["ca_Wk"][:, perm]; ca_bk = f["ca_bk"][perm]
    ca_Wv = f["ca_Wv"][:, perm]; ca_bv = f["ca_bv"][perm]
    W1 = _blk(f["mlp_W1"][perm, :])
    W2 = _blk(f["mlp_W2"][:, perm])
    b2p = f["mlp_b2"][perm]

    def pcol(v):  # [X] -> [128, X//128] block-major per-partition layout
        return np.ascontiguousarray(v.reshape(-1, PB).T)

    in_maps = []
    for c in range(NCORES):
        b, r = c // GSZ, c % GSZ
        sl = slice(r * SH, (r + 1) * SH)
        pkf = np.concatenate(
            [pcol(sa_bq[sl]), pcol(sa_bk[sl]), pcol(ca_bq[sl]), pcol(ca_bk[sl]),
             pcol(f["mlp_b1"]), pcol(b2p),
             pcol(f["ln1_g"][perm]), pcol(f["ln1_b"][perm]),
             pcol(f["ln2_g"][perm]), pcol(f["ln2_b"][perm]),
             pcol(f["ln3_g"][perm]), pcol(f["ln3_b"][perm])], axis=1)
        pkd = np.concatenate(
            [np.ones((PB, HPC), np.float32), np.ones((PB, PB), np.float32)],
            axis=1)
        pkrow = np.concatenate(
            [sa_bv[sl], ca_bv[sl], np.ones(PB, np.float32),
             np.ones(HD, np.float32)])[None, :]
        m = {
            "xT": _blk(np.ascontiguousarray(f["x"][b][:, perm].T)),
            "ctxT": _blk(np.ascontiguousarray(f["context"][b].T)),
            "wq": _blk(sa_Wq[:, sl]), "wk": _blk(sa_Wk[:, sl]),
            "wv": _blk(sa_Wv[:, sl]),
            "cwq": _blk(ca_Wq[:, sl]), "cwk": _blk(ca_Wk[:, sl]),
            "cwv": _blk(ca_Wv[:, sl]),
            "w1": W1, "w2": W2,
            "pk_f32": np.ascontiguousarray(pkf),
            "pk_dt": np.ascontiguousarray(pkd),
            "pk_row": np.ascontiguousarray(pkrow),
        }
        m = {k: (v if k == "pk_f32" else
                 np.ascontiguousarray(v.astype(bf16)))
             for k, v in m.items()}
        in_maps.append(m)
    return in_maps, inv


def kernel(**inputs) -> np.ndarray:
    global _COMPILED
    if _COMPILED is None:
        _COMPILED = _build()
    nc = _COMPILED
    in_maps, inv = _host_prep(inputs)
    res = run_bass_kernel_spmd(nc, in_maps, list(range(NCORES)), trace=TRACE)
    kernel.last_results = res
    out = np.empty((B, P, E), np.float32)
    for c in range(NCORES):
        b, r = c // GSZ, c % GSZ
        o = res.results[c]["outT"]          # [128, 8, 256]
        out[b, r * TS:(r + 1) * TS, :] = o.transpose(1, 0, 2).reshape(E, TS).T
    return np.ascontiguousarray(out[:, :, inv])



# revision 15
# speedup vs baseline: 1.2146x; 1.1616x over previous
"""Trainium2 Bass kernel for nn_DecoderLayer (dense transformer decoder layer).

Full inputs in, full output out. Internally sharded across 8 NeuronCores:
  core c (0..7): batch b = c//4, group rank r = c%4
  - features permuted so heads are contiguous: perm[h*64+d] = d*16+h
  - self/cross attention: head-parallel, 4 heads per core (256 permuted cols)
  - AllGather of attention outputs over each 4-core group
  - layernorm computed redundantly per group (transposed layout, partition
    reductions via ones-matmuls)
  - MLP + final LN: token-parallel, 256 tokens per core, assembled on host

All activations live in transposed layout [feature, token]; weights are
pre-permuted/sliced/scaled on the host and pre-blocked to [128, 8, w] so each
load is a single DMA with 8-32KB contiguous per-partition lines. Matmuls run
in float32r (full rate on TRN2 for free dim >= 256).
"""
import sys

if "/opt/trn_rl_repo" not in sys.path:
    sys.path.insert(0, "/opt/trn_rl_repo")

import numpy as np

import concourse.bass as bass
import concourse.mybir as mybir
import concourse.tile as tile
from concourse import bacc
from concourse.bass import ds
from concourse.bass_utils import run_bass_kernel_spmd

# ---- problem constants (hardcoded per contract) ----
E = 1024          # embed dim
H = 1024          # hidden dim
NH = 16           # heads
HD = 64           # head dim
B = 2             # batch
P = 1024          # problem (query) length
C = 1024          # context length
NCORES = 8
GSZ = 4           # cores per batch group
HPC = NH // GSZ   # heads per core = 4
SH = HPC * HD     # per-core head shard width = 256
TS = P // GSZ     # per-core token slice for MLP = 256
PB = 128          # partitions
EB = E // PB      # 8 feature blocks

DT = mybir.dt.bfloat16
F32 = mybir.dt.float32
Exp = mybir.ActivationFunctionType.Exp
Sqrt = mybir.ActivationFunctionType.Sqrt
ADD = mybir.AluOpType.add
MULT = mybir.AluOpType.mult
SUB = mybir.AluOpType.subtract
MAX = mybir.AluOpType.max

_COMPILED = None
TRACE = False      # test harness may flip this


def _build():
    nc = bacc.Bacc(trn_type="TRN2", num_devices=NCORES,
                   target_bir_lowering=False, debug=False)

    def din(name, shape, dt=DT):
        return nc.dram_tensor(name, shape, dt, kind="ExternalInput").ap()

    # big inputs, host pre-blocked to [128, EB, w]
    xT = din("xT", [PB, EB, P])
    ctxT = din("ctxT", [PB, EB, C])
    wq = din("wq", [PB, EB, SH]); wk = din("wk", [PB, EB, SH])
    wv = din("wv", [PB, EB, SH])
    cwq = din("cwq", [PB, EB, SH]); cwk = din("cwk", [PB, EB, SH])
    cwv = din("cwv", [PB, EB, SH])
    w1 = din("w1", [PB, EB, H]); w2 = din("w2", [PB, EB, E])
    # packed params: [128, 8+64] f32 = bq|bk|cbq|cbk (2 each) then
    # b1|b2|g1|be1|g2|be2|g3|be3 (8 each)
    pk_f32 = din("pk_f32", [PB, 72], F32)
    # packed f32r col params: ones_c4 (4) | ones_128 (128)
    pk_dt = din("pk_dt", [PB, 132])
    # packed f32r row params: bvr(256)|cbvr(256)|ones_r128(128)|ones_r64(64)
    pk_row = din("pk_row", [1, 704])

    outT = nc.dram_tensor("outT", [PB, EB, TS], F32, kind="ExternalOutput").ap()

    cc1_in = [nc.dram_tensor(f"cc1_in{t}", [SH, 512], DT).ap() for t in range(2)]
    cc1_out = [nc.dram_tensor(f"cc1_out{t}", [E, 512], DT).ap() for t in range(2)]
    cc2_in = [nc.dram_tensor(f"cc2_in{t}", [SH, 512], DT).ap() for t in range(2)]
    cc2_all = nc.dram_tensor("cc2_all", [2 * E, 512], DT).ap()
    h1s_dram = nc.dram_tensor("h1s_dram", [E, TS], DT).ap()

    groups = [[0, 1, 2, 3], [4, 5, 6, 7]]

    with tile.TileContext(nc) as tc, \
         nc.allow_low_precision(reason="f32r rounding of matmul operands"):
        pid = nc.partition_id()
        off = (pid % GSZ) * TS            # token slice start for MLP
        tci = (pid % GSZ) // 2 * E        # AG2 chunk row base (0 or 1024)
        colo = (pid % 2) * TS             # column offset inside AG2 chunk

        # ---------- persistent pools (left stack bottom) ----------
        prm = tc.alloc_tile_pool(name="prm", bufs=1)
        pp = tc.alloc_tile_pool(name="pp", bufs=2, space="PSUM")
        sp = tc.alloc_tile_pool(name="sp", bufs=2, space="PSUM")
        op = tc.alloc_tile_pool(name="op", bufs=2, space="PSUM")
        epl = tc.alloc_tile_pool(name="epl", bufs=3)
        upl = tc.alloc_tile_pool(name="upl", bufs=5)
        rcp = tc.alloc_tile_pool(name="rcp", bufs=4)
        rdt = tc.alloc_tile_pool(name="rdt", bufs=2)
        sqp = tc.alloc_tile_pool(name="sqp", bufs=9)
        ltp = tc.alloc_tile_pool(name="ltp", bufs=3)
        lkp = tc.alloc_tile_pool(name="lkp", bufs=4)
        PERSIST = (prm, pp, sp, op, epl, upl, rcp, rdt, sqp, ltp, lkp)

        pkf = prm.tile([PB, 72], F32, tag="pkf", name="pkf")
        nc.gpsimd.dma_start(pkf[:], pk_f32[:])
        pkd = prm.tile([PB, 132], DT, tag="pkd", name="pkd")
        nc.gpsimd.dma_start(pkd[:], pk_dt[:])
        pkr = prm.tile([1, 704], DT, tag="pkr", name="pkr")
        nc.gpsimd.dma_start(pkr[:], pk_row[:])

        bq_s, bk_s, cbq_s, cbk_s = (pkf[:, 2 * i:2 * i + 2] for i in range(4))
        (b1_s, b2_s, g1_s, be1_s, g2_s, be2_s, g3_s, be3_s) = (
            pkf[:, 8 + 8 * i:16 + 8 * i] for i in range(8))
        o_c4 = pkd[:, 0:HPC]
        o_128 = pkd[:, HPC:HPC + PB]
        bvr_s = pkr[:, 0:SH]
        cbvr_s = pkr[:, SH:2 * SH]
        o_r128 = pkr[:, 2 * SH:2 * SH + PB]
        o_r64 = pkr[:, 2 * SH + PB:2 * SH + PB + HD]

        # ============ helpers ============
        def proj_qk(out_pool, out_tag, w_t, rhs_t, bias_sb):
            """2 blocks [128, 1024] = (W.T @ actT + bias)."""
            outs = [out_pool.tile([PB, P], DT, tag=out_tag, name=f"{out_tag}_{i}")
                    for i in range(2)]
            for mb in range(2):
                for tcx in range(2):
                    ps = pp.tile([PB, 512], F32, tag="pp", name="ps_proj")
                    for eb in range(EB):
                        nc.tensor.matmul(
                            ps[:], w_t[:, eb, mb * PB:(mb + 1) * PB],
                            rhs_t[:, eb, tcx * 512:(tcx + 1) * 512],
                            start=(eb == 0), stop=(eb == EB - 1))
                    nc.vector.tensor_scalar(
                        outs[mb][:, tcx * 512:(tcx + 1) * 512], ps[:],
                        bias_sb[:, mb:mb + 1], None, ADD)
            return outs

        def proj_v(v_pool, v_tag, act_t, wv_t, bvrow):
            """v tiles: 8 x [128, HPC, HD+1]; [:, h, :HD] = (act @ Wv + bv),
            [:, h, HD] = 1.0 (softmax-sum column)."""
            vts = []
            for jb in range(EB):
                ps = pp.tile([PB, SH], F32, tag="pp", name="ps_v")
                nc.tensor.matmul(ps[:], o_r128, bvrow, start=True, stop=False)
                for eb in range(EB):
                    nc.tensor.matmul(
                        ps[:], act_t[:, eb, jb * PB:(jb + 1) * PB],
                        wv_t[:, eb, :], start=False, stop=(eb == EB - 1))
                vt = v_pool.tile([PB, HPC, HD + 1], DT, tag=v_tag,
                                 name=f"{v_tag}_{jb}")
                for h in range(HPC):
                    nc.vector.tensor_copy(vt[:, h, 0:HD],
                                          ps[:, h * HD:(h + 1) * HD])
                nc.gpsimd.dma_start(vt[:, :, HD:HD + 1], o_c4[:, :, None])
                vts.append(vt)
            return vts

        def attention(qt, kt, vts, pa, tcx):
            """One 512-token chunk of 4-head attention; writes pa[hp][:, chunk].
            Software-pipelined: scores/exp for jb+1 are emitted before the
            accumulate matmuls of jb so the PE never waits on the exp."""
            tsl = slice(tcx * 512, (tcx + 1) * 512)
            us = []
            for hp in range(2):
                accs = [op.tile([HD + 1, 512], F32, tag="op", name=f"acc{i}")
                        for i in range(2)]
                exs = {}
                for jb in range(EB + 1):
                    if jb < EB:
                        s = sp.tile([PB, 1024], F32, tag="sp", name="s_scores")
                        for hi in range(2):
                            nc.tensor.matmul(
                                s[:, hi * 512:(hi + 1) * 512],
                                kt[hp][hi * HD:(hi + 1) * HD, jb * PB:(jb + 1) * PB],
                                qt[hp][hi * HD:(hi + 1) * HD, tsl],
                                start=True, stop=True)
                        ex = epl.tile([PB, 1024], DT, tag="ep", name="ex")
                        nc.scalar.activation(ex[:], s[:], Exp)
                        exs[jb] = ex
                    if jb > 0:
                        pj = jb - 1
                        ex = exs.pop(pj)
                        for hi in range(2):
                            nc.tensor.matmul(
                                accs[hi], vts[pj][:, 2 * hp + hi, :],
                                ex[:, hi * 512:(hi + 1) * 512],
                                start=(pj == 0), stop=(pj == EB - 1))
                for hi in range(2):
                    u = upl.tile([HD + 1, 512], F32, tag="up", name="u_out")
                    nc.scalar.copy(u[:], accs[hi])
                    rec_dt = rdt.tile([1, 512], DT, tag="recdt", name="rec_dt")
                    nc.vector.reciprocal(rec_dt[:], u[HD:HD + 1, :])
                    bc = pp.tile([HD, 512], F32, tag="pp", name="bc")
                    nc.tensor.matmul(bc[:], o_r64, rec_dt[:],
                                     start=True, stop=True)
                    nc.vector.tensor_tensor(
                        pa[hp][hi * HD:(hi + 1) * HD, tsl], u[0:HD, :],
                        bc[:], MULT)

        def ln_chunk(a_of, g_sb, b_sb, resid_fn, out_of, w):
            """LN over features (partitions) of a_of(eb) (8 x [128, w]) + affine
            + residual -> out_of(eb). Destroys a_of slices."""
            s1 = pp.tile([PB, w], F32, tag="pp", name="s1")
            s2 = pp.tile([PB, w], F32, tag="pp", name="s2")
            sqs = []
            for eb in range(EB):
                sq = sqp.tile([PB, w], DT, tag="sq", name="sq")
                nc.vector.tensor_tensor(sq[:], a_of(eb), a_of(eb), MULT)
                sqs.append(sq)
                nc.tensor.matmul(s1[:], o_128, a_of(eb),
                                 start=(eb == 0), stop=(eb == EB - 1))
            for eb in range(EB):
                nc.tensor.matmul(s2[:], o_128, sqs[eb][:],
                                 start=(eb == 0), stop=(eb == EB - 1))
            mean = ltp.tile([PB, w], F32, tag="lt", name="mean")
            nc.scalar.mul(mean[:], s1[:], 1.0 / E)
            msq = ltp.tile([PB, w], F32, tag="lt", name="msq")
            nc.vector.tensor_tensor(msq[:], mean[:], mean[:], MULT)
            nc.vector.tensor_scalar(msq[:], msq[:], -1e-5, None, ADD)
            var = ltp.tile([PB, w], F32, tag="lt", name="var")
            nc.vector.scalar_tensor_tensor(var[:], s2[:], 1.0 / E, msq[:], MULT, SUB)
            rstd = lkp.tile([PB, w], F32, tag="lk", name="rstd")
            nc.scalar.activation(rstd[:], var[:],
                                 mybir.ActivationFunctionType.Abs_reciprocal_sqrt)
            nmr = lkp.tile([PB, w], F32, tag="lk", name="nmr")
            nc.vector.scalar_tensor_tensor(nmr[:], mean[:], -1.0, rstd[:], MULT, MULT)
            for eb in range(EB):
                a = a_of(eb)
                nc.vector.tensor_tensor(a, a, rstd[:], MULT)
                nc.vector.tensor_tensor(a, a, nmr[:], ADD)
                nc.vector.tensor_scalar(a, a, g_sb[:, eb:eb + 1],
                                        b_sb[:, eb:eb + 1], MULT, ADD)
                nc.vector.tensor_tensor(out_of(eb), a, resid_fn(eb), ADD)

        # ============ stage 1: big loads ============
        with nc.named_scope("loads"):
            xt_pool = tc.alloc_tile_pool(name="xt", bufs=1)
            xt = xt_pool.tile([PB, EB, P], DT, tag="xt", name="xt")
            for eb in range(EB):
                nc.sync.dma_start(xt[:, eb, :], xT[:, eb, :])
            wsa_pool = tc.alloc_tile_pool(name="wsa", bufs=3)
            wq_t = wsa_pool.tile([PB, EB, SH], DT, tag="wsa", name="wq_t")
            nc.gpsimd.dma_start(wq_t[:], wq[:])
            wk_t = wsa_pool.tile([PB, EB, SH], DT, tag="wsa", name="wk_t")
            nc.gpsimd.dma_start(wk_t[:], wk[:])
            wv_t = wsa_pool.tile([PB, EB, SH], DT, tag="wsa", name="wv_t")
            nc.gpsimd.dma_start(wv_t[:], wv[:])

        # right stack: pa at the bottom
        pa_pool = tc.alloc_tile_pool(name="pa", bufs=2, side="right")
        pa = [pa_pool.tile([PB, P], DT, tag="pa", name=f"pa{i}") for i in range(2)]
        qk1_pool = tc.alloc_tile_pool(name="qk1", bufs=4, side="right")
        v1_pool = tc.alloc_tile_pool(name="v1", bufs=EB, side="right")

        # ============ stage 2: SA projections ============
        with nc.named_scope("sa_proj"):
            qt = proj_qk(qk1_pool, "qk1", wq_t, xt, bq_s)
            kt = proj_qk(qk1_pool, "qk1", wk_t, xt, bk_s)
            vts = proj_v(v1_pool, "v1", xt, wv_t, bvr_s)
        wsa_pool.release()

        # ctx load (DMA overlaps SA attention)
        ctx_pool = tc.alloc_tile_pool(name="ctx", bufs=1)
        ctx = ctx_pool.tile([PB, EB, C], DT, tag="ctx", name="ctx")
        for cb in range(2):
            nc.sync.dma_start(ctx[:, 4 * cb:4 * (cb + 1), :],
                              ctxT[:, 4 * cb:4 * (cb + 1), :])

        # ============ stage 3: SA attention + chunked AllGather ============
        with nc.named_scope("sa_attn"):
            for tcx in range(2):
                attention(qt, kt, vts, pa, tcx)
                tsl = slice(tcx * 512, (tcx + 1) * 512)
                for mb in range(2):
                    nc.sync.dma_start(cc1_in[tcx][mb * PB:(mb + 1) * PB, :],
                                      pa[mb][:, tsl])
                nc.gpsimd.collective_compute(
                    "AllGather", mybir.AluOpType.bypass, replica_groups=groups,
                    ins=[cc1_in[tcx][:]], outs=[cc1_out[tcx][:]])
        v1_pool.release()
        qk1_pool.release()

        # ============ stage 4: CA weight loads + k/v projections ============
        qk2_pool = tc.alloc_tile_pool(name="qk2", bufs=4, side="right")
        v2_pool = tc.alloc_tile_pool(name="v2", bufs=EB, side="right")
        wca_pool = tc.alloc_tile_pool(name="wca", bufs=3, side="right")
        with nc.named_scope("ca_kv"):
            cwk_t = wca_pool.tile([PB, EB, SH], DT, tag="wca", name="cwk_t")
            nc.gpsimd.dma_start(cwk_t[:], cwk[:])
            cwv_t = wca_pool.tile([PB, EB, SH], DT, tag="wca", name="cwv_t")
            nc.gpsimd.dma_start(cwv_t[:], cwv[:])
            cwq_t = wca_pool.tile([PB, EB, SH], DT, tag="wca", name="cwq_t")
            nc.gpsimd.dma_start(cwq_t[:], cwq[:])
            k2t = proj_qk(qk2_pool, "qk2", cwk_t, ctx, cbk_s)
            v2ts = proj_v(v2_pool, "v2", ctx, cwv_t, cbvr_s)
        ctx_pool.release()

        # ============ stage 5: LN1 + residual -> h1 ============
        h1_pool = tc.alloc_tile_pool(name="h1", bufs=1, side="right")
        h1 = h1_pool.tile([PB, EB, P], DT, tag="h1", name="h1")
        a1_pool = tc.alloc_tile_pool(name="a1", bufs=1, side="right")
        with nc.named_scope("ln1"):
            for tcx in range(2):
                a1 = a1_pool.tile([PB, EB, 512], DT, tag="a1", name="a1")
                for eb in range(EB):
                    nc.gpsimd.dma_start(
                        a1[:, eb, :], cc1_out[tcx][eb * PB:(eb + 1) * PB, :])
                tsl = slice(tcx * 512, (tcx + 1) * 512)
                ln_chunk(lambda eb: a1[:, eb, :], g1_s, be1_s,
                         lambda eb: xt[:, eb, tsl],
                         lambda eb: h1[:, eb, tsl], 512)
            # stash my token slice of h1 for the LN2 residual
            nc.sync.dma_start(
                h1s_dram.rearrange("(e p) n -> p e n", p=PB),
                h1[:, :, ds(off, TS)])
        a1_pool.release()
        xt_pool.release()

        # ============ stage 6: CA q projection + attention ============
        with nc.named_scope("ca_q"):
            qt2 = proj_qk(qk2_pool, "qk2", cwq_t, h1, cbq_s)
        h1_pool.release()
        wca_pool.release()

        # left stack: tail pools (w1/w2 DMAs overlap CA attention)
        out_pool = tc.alloc_tile_pool(name="outp", bufs=1)
        y_pool = tc.alloc_tile_pool(name="yp", bufs=1)
        h2_pool = tc.alloc_tile_pool(name="h2", bufs=1)
        w2_pool = tc.alloc_tile_pool(name="w2p", bufs=1)
        m1_pool = tc.alloc_tile_pool(name="m1", bufs=1)
        w1_pool = tc.alloc_tile_pool(name="w1p", bufs=1)
        rtp = tc.alloc_tile_pool(name="rtp", bufs=1)
        a2_pool = tc.alloc_tile_pool(name="a2", bufs=1)
        w1_t = w1_pool.tile([PB, EB, H], DT, tag="w1", name="w1_t")
        nc.gpsimd.dma_start(w1_t[:], w1[:])
        w2_t = w2_pool.tile([PB, EB, E], DT, tag="w2", name="w2_t")
        nc.gpsimd.dma_start(w2_t[:], w2[:])

        with nc.named_scope("ca_attn"):
            for tcx in range(2):
                attention(qt2, k2t, v2ts, pa, tcx)
                tsl = slice(tcx * 512, (tcx + 1) * 512)
                for mb in range(2):
                    nc.sync.dma_start(cc2_in[tcx][mb * PB:(mb + 1) * PB, :],
                                      pa[mb][:, tsl])
                nc.gpsimd.collective_compute(
                    "AllGather", mybir.AluOpType.bypass, replica_groups=groups,
                    ins=[cc2_in[tcx][:]],
                    outs=[cc2_all[tcx * E:(tcx + 1) * E, :]])
        v2_pool.release()
        qk2_pool.release()
        pa_pool.release()

        # ============ stage 7: LN2 + residual on my token slice -> h2 ============
        with nc.named_scope("ln2_mlp"):
            h2 = h2_pool.tile([PB, EB, TS], DT, tag="h2", name="h2")
            a2 = a2_pool.tile([PB, EB, TS], DT, tag="a2", name="a2")
            for eb in range(EB):
                nc.gpsimd.dma_start(
                    a2[:, eb, :],
                    cc2_all[ds(tci + eb * PB, PB), ds(colo, TS)])
            rt = rtp.tile([PB, EB, TS], DT, tag="rt", name="rt")
            for eb in range(EB):
                nc.gpsimd.dma_start(rt[:, eb, :],
                                    h1s_dram[eb * PB:(eb + 1) * PB, :])
            ln_chunk(lambda eb: a2[:, eb, :], g2_s, be2_s,
                     lambda eb: rt[:, eb, :],
                     lambda eb: h2[:, eb, :], TS)
            a2_pool.release()
            rtp.release()

            # ============ stage 8: MLP ============
            m1 = m1_pool.tile([PB, EB, TS], DT, tag="m1", name="m1")
            for hb in range(EB):
                ps = pp.tile([PB, TS], F32, tag="pp", name="ps_m1")
                for eb in range(EB):
                    nc.tensor.matmul(ps[:], w1_t[:, eb, hb * PB:(hb + 1) * PB],
                                     h2[:, eb, :], start=(eb == 0),
                                     stop=(eb == EB - 1))
                nc.vector.tensor_scalar(m1[:, hb, :], ps[:], b1_s[:, hb:hb + 1],
                                        0.0, ADD, MAX)
            w1_pool.release()

            y = y_pool.tile([PB, EB, TS], DT, tag="y", name="y")
            for eb in range(EB):
                ps = pp.tile([PB, TS], F32, tag="pp", name="ps_y")
                for hb in range(EB):
                    nc.tensor.matmul(ps[:], w2_t[:, hb, eb * PB:(eb + 1) * PB],
                                     m1[:, hb, :], start=(hb == 0),
                                     stop=(hb == EB - 1))
                nc.vector.tensor_scalar(y[:, eb, :], ps[:], b2_s[:, eb:eb + 1],
                                        None, ADD)
            m1_pool.release()
            w2_pool.release()

        # ============ stage 9: LN3 + residual -> output ============
        with nc.named_scope("ln3"):
            ot = out_pool.tile([PB, EB, TS], F32, tag="ot", name="ot")
            ln_chunk(lambda eb: y[:, eb, :], g3_s, be3_s,
                     lambda eb: h2[:, eb, :],
                     lambda eb: ot[:, eb, :], TS)
            nc.sync.dma_start(outT[:], ot[:])
        h2_pool.release()
        y_pool.release()
        out_pool.release()
        for _pl in reversed(PERSIST):
            _pl.release()

    nc.compile()
    return nc


def _blk(M):
    """[1024, w] -> [128, 8, w] partition-blocked contiguous."""
    return np.ascontiguousarray(M.reshape(EB, PB, -1).transpose(1, 0, 2))


def _host_prep(inputs):
    import ml_dtypes
    bf16 = ml_dtypes.bfloat16
    f = {k: np.ascontiguousarray(np.asarray(v, dtype=np.float32))
         for k, v in inputs.items()}
    perm = np.empty(E, dtype=np.int64)
    for h in range(NH):
        for d in range(HD):
            perm[h * HD + d] = d * NH + h
    inv = np.argsort(perm)
    s = np.float32(1.0 / np.sqrt(HD))

    sa_Wq = f["sa_Wq"][perm][:, perm] * s
    sa_bq = f["sa_bq"][perm] * s
    sa_Wk = f["sa_Wk"][perm][:, perm]; sa_bk = f["sa_bk"][perm]
    sa_Wv = f["sa_Wv"][perm][:, perm]; sa_bv = f["sa_bv"][perm]
    ca_Wq = f["ca_Wq"][perm][:, perm] * s
    ca_bq = f["ca_bq"][perm] * s
    ca_Wk = f["ca_Wk"][:, perm]; ca_bk = f["ca_bk"][perm]
    ca_Wv = f["ca_Wv"][:, perm]; ca_bv = f["ca_bv"][perm]
    W1 = _blk(f["mlp_W1"][perm, :])
    W2 = _blk(f["mlp_W2"][:, perm])
    b2p = f["mlp_b2"][perm]

    def pcol(v):  # [X] -> [128, X//128] block-major per-partition layout
        return np.ascontiguousarray(v.reshape(-1, PB).T)

    in_maps = []
    for c in range(NCORES):
        b, r = c // GSZ, c % GSZ
        sl = slice(r * SH, (r + 1) * SH)
        pkf = np.concatenate(
            [pcol(sa_bq[sl]), pcol(sa_bk[sl]), pcol(ca_bq[sl]), pcol(ca_bk[sl]),
             pcol(f["mlp_b1"]), pcol(b2p),
             pcol(f["ln1_g"][perm]), pcol(f["ln1_b"][perm]),
             pcol(f["ln2_g"][perm]), pcol(f["ln2_b"][perm]),
             pcol(f["ln3_g"][perm]), pcol(f["ln3_b"][perm])], axis=1)
        pkd = np.concatenate(
            [np.ones((PB, HPC), np.float32), np.ones((PB, PB), np.float32)],
            axis=1)
        pkrow = np.concatenate(
            [sa_bv[sl], ca_bv[sl], np.ones(PB, np.float32),
             np.ones(HD, np.float32)])[None, :]
        m = {
            "xT": _blk(np.ascontiguousarray(f["x"][b][:, perm].T)),
            "ctxT": _blk(np.ascontiguousarray(f["context"][b].T)),
            "wq": _blk(sa_Wq[:, sl]), "wk": _blk(sa_Wk[:, sl]),
            "wv": _blk(sa_Wv[:, sl]),
            "cwq": _blk(ca_Wq[:, sl]), "cwk": _blk(ca_Wk[:, sl]),
            "cwv": _blk(ca_Wv[:, sl]),
            "w1": W1, "w2": W2,
            "pk_f32": np.ascontiguousarray(pkf),
            "pk_dt": np.ascontiguousarray(pkd),
            "pk_row": np.ascontiguousarray(pkrow),
        }
        m = {k: (v if k == "pk_f32" else
                 np.ascontiguousarray(v.astype(bf16)))
             for k, v in m.items()}
        in_maps.append(m)
    return in_maps, inv


def kernel(**inputs) -> np.ndarray:
    global _COMPILED
    if _COMPILED is None:
        _COMPILED = _build()
    nc = _COMPILED
    in_maps, inv = _host_prep(inputs)
    res = run_bass_kernel_spmd(nc, in_maps, list(range(NCORES)), trace=TRACE)
    kernel.last_results = res
    out = np.empty((B, P, E), np.float32)
    for c in range(NCORES):
        b, r = c // GSZ, c % GSZ
        o = res.results[c]["outT"]          # [128, 8, 256]
        out[b, r * TS:(r + 1) * TS, :] = o.transpose(1, 0, 2).reshape(E, TS).T
    return np.ascontiguousarray(out[:, :, inv])

